# revision 5
# baseline (speedup 1.0000x reference)
"""BERT self-attention (B=4, S=1024, H=1024, 16 heads, d=64) on 8 TRN2 cores.

Sharding: core c = b*2 + g handles batch b and head-group g (8 heads, 512
output columns).  No cross-core communication.

Baseline was 140.2us with PE (matmul pipe) busy 116.4us, a 17us dead start
(serialized DMA issues on 2 queues, W ct0 slices queued behind misc loads)
and a 12.6us tail (drain of ~29 DMA semaphores + fp32 transposes).

This version:
  1. Startup: input DMAs spread over 3 HWDGE queues (sync/scalar/vector)
     ordered by first use; wq|wk ct0 packed into one 512B-row tensor so the
     first QTKT chain starts ~9-10us in.  Six warmup matmuls on a memset
     tile run during the DMA fill so the PE p-state ramp (0.65/1.2GHz ->
     2.4GHz over ~3us) is spent before real work; EXP activation table is
     preloaded off the critical path.
  2. Tail: bq/bk packed into one tensor, context staged into a persistent
     [128, 8, 512] SBUF tile and written with 2 output DMAs (512B runs,
     1x DMA cost) instead of 8, fewer semaphores for the end drain.
  3. f16 normalization path: ctx~^T copied to f16, PE transpose in f16
     (1 cycle/row vs 2 for fp32), f16 PSUM transpose output lets the DVE
     scalar-mul run in 2x mode.

Per-core dataflow (matmul inputs fp16, accumulation fp32 PSUM):
  scores^T[k, q] = K_h^T.T @ Q_h^T (exp on ACT, 1/8 scale folded in); Vaug
  carries a ones column so ctx~^T = Vaug^T P^T also yields softmax
  denominators; PE-transpose back to [q, d+1], per-partition reciprocal *
  tensor_scalar_mul into the staging tile.  Software-pipelined head loop
  keeps scores one head ahead of ctx so the ACT exp stream (~71us) hides
  under PE work (~113us busy).
"""

import numpy as np

B, S, H = 4, 1024, 1024
NH, D = 16, 64
NCORES = 8
HG = NH // 2        # heads per core
CW = HG * D         # output columns per core (512)
P = 128             # partitions

_CACHE = {}


def _split_excess_waits(nc, mybir):
    """Walrus codegen allows 1 sync-wait per instruction (2 for
    EventSemaphore); Tile's tail drain (and some matmuls) carry more.
    Move the excess onto NoOp carriers inserted just before, same engine."""
    for f in nc.m.functions:
        for bb in f.blocks:
            new_insts, changed = [], False
            for inst in bb.instructions:
                si = inst.sync_info
                cap = 2 if inst.opcode == "EventSemaphore" else 1
                if si is not None and si.on_wait and len(si.on_wait) > cap:
                    waits = list(si.on_wait)
                    for i, w in enumerate(waits[:-cap]):
                        nop = mybir.InstNoOp(
                            name=f"{inst.name}-wsplit{i}",
                            engine=inst.engine,
                            sync_info=mybir.SyncInfo(on_wait=[w], on_update=[]),
                            bass_nofuse=True,
                        )
                        nc.register_instruction(nop, overwrite=True)
                        new_insts.append(nop)
                    inst.sync_info = mybir.SyncInfo(
                        on_wait=waits[-cap:],
                        on_update=list(si.on_update or []))
                    changed = True
                new_insts.append(inst)
            if changed:
                bb.instructions = new_insts


def _build():
    import concourse.bass as bass
    import concourse.mybir as mybir
    import concourse.tile as tile
    from contextlib import ExitStack

    f32 = mybir.dt.float32
    f16 = mybir.dt.float16
    EXP = mybir.ActivationFunctionType.Exp

    nc = bass.Bass()
    x_d = nc.dram_tensor("x", [H, S], f16, kind="ExternalInput")      # X^T
    wqk0_d = nc.dram_tensor("wqk0", [H, 2 * P], f16, kind="ExternalInput")
    wqr_d = nc.dram_tensor("wqr", [H, 3 * P], f16, kind="ExternalInput")
    wkr_d = nc.dram_tensor("wkr", [H, 3 * P], f16, kind="ExternalInput")
    wv_d = nc.dram_tensor("wv", [H, CW], f16, kind="ExternalInput")
    bqk_d = nc.dram_tensor("bqk", [P, 8], f32, kind="ExternalInput")
    bvb_d = nc.dram_tensor("bvb", [P, CW], f32, kind="ExternalInput")
    id16_d = nc.dram_tensor("id16", [P, P], f16, kind="ExternalInput")
    out_d = nc.dram_tensor("out", [S, CW], f16, kind="ExternalOutput")

    with tile.TileContext(nc) as tc, ExitStack() as ctx:
        persist = ctx.enter_context(tc.tile_pool(name="persist", bufs=1))
        ptpool = ctx.enter_context(tc.tile_pool(name="ptpool", bufs=2))
        ctspool = ctx.enter_context(tc.tile_pool(name="ctspool", bufs=2))
        rpool = ctx.enter_context(tc.tile_pool(name="rpool", bufs=4))
        pss = ctx.enter_context(tc.tile_pool(name="pss", bufs=2, space="PSUM"))
        psc = ctx.enter_context(tc.tile_pool(name="psc", bufs=2, space="PSUM"))
        pst = ctx.enter_context(tc.tile_pool(name="pst", bufs=2, space="PSUM"))

        wqk0_s = persist.tile([P, 8, 2 * P], f16, tag="wqk0")
        wqr_s = persist.tile([P, 8, 3 * P], f16, tag="wqr")
        wkr_s = persist.tile([P, 8, 3 * P], f16, tag="wkr")
        wv_s = persist.tile([P, 8, CW], f16, tag="wv")
        ident = persist.tile([P, P], f16, tag="ident")
        xt = persist.tile([P, 8, S], f16, tag="xt")          # X^T [h, hc, s]
        qt = persist.tile([P, 4, S], f16, tag="qt")          # Q^T [col, ct, s]
        kt = persist.tile([P, 4, S], f16, tag="kt")          # K^T
        vaug = persist.tile([P, 8, HG, D + 1], f16, tag="vaug")  # V + ones col
        bqk = persist.tile([P, 8], f32, tag="bqk")
        bvb = persist.tile([P, CW], f32, tag="bvb")          # bv broadcast
        ones1 = persist.tile([1, P], f32, tag="ones1")
        onesf = persist.tile([P, 8, HG], f32, tag="onesf")
        warm = persist.tile([P, 512], f16, tag="warm")
        actw = persist.tile([P, 1], f32, tag="actw")
        out_s = persist.tile([P, 8, CW], f16, tag="outs")    # ctx staging

        # ---- warmup: memset a tile, then 6 matmuls so the PE p-state
        # ramp completes while the input DMAs stream in ----
        nc.gpsimd.memset(warm, 0.0625)
        for _ in range(6):
            ps = psc.tile([P, 512], f32, tag="psc")
            nc.tensor.matmul(ps, lhsT=warm[:, 0:P], rhs=warm,
                             start=True, stop=True)

        # ---- input DMAs: 3 HWDGE queues, ordered by first use ----
        x_r = x_d.rearrange("(c p) s -> p c s", p=P)
        wqr_r = wqr_d.rearrange("(c p) n -> p c n", p=P)
        wkr_r = wkr_d.rearrange("(c p) n -> p c n", p=P)
        # sync: first X chunks (first QTKT chain consumes in hcc order)
        for hc in (0, 1, 2, 3):
            nc.sync.dma_start(out=xt[:, hc, :], in_=x_r[:, hc, :])
        # scalar: packed wq|wk ct0 (needed by the first chain), remaining X
        # chunks, wv, then EXP table preload; ACT's sequencer is free again
        # before the exp stream starts
        nc.scalar.dma_start(out=wqk0_s,
                            in_=wqk0_d.rearrange("(c p) n -> p c n", p=P))
        for hc in (4, 5, 6, 7):
            nc.scalar.dma_start(out=xt[:, hc, :], in_=x_r[:, hc, :])
        nc.scalar.dma_start(out=wv_s, in_=wv_d.rearrange("(c p) n -> p c n", p=P))
        nc.scalar.activation(actw, warm[:, 0:1], EXP, scale=1.0)
        # gpsimd (SWDGE): everything needed after ~15us
        nc.gpsimd.dma_start(out=bqk, in_=bqk_d[:, :])
        nc.gpsimd.dma_start(out=bvb, in_=bvb_d[:, :])
        nc.gpsimd.dma_start(out=ident, in_=id16_d[:, :])
        nc.gpsimd.dma_start(out=wqr_s, in_=wqr_r)
        nc.gpsimd.dma_start(out=wkr_s, in_=wkr_r)

        nc.vector.memset(onesf, 1.0)
        nc.vector.tensor_copy(vaug[:, :, :, D], onesf)
        nc.vector.memset(ones1, 1.0)

        def emit_qtkt(ct):
            for wi, dst in ((0, qt), (1, kt)):
                for sb in range(2):
                    ps = psc.tile([P, 512], f32, tag="psc")
                    for hcc in range(8):
                        if ct == 0:
                            w_ap = wqk0_s[:, hcc, wi * P:(wi + 1) * P]
                        else:
                            w_s = wqr_s if wi == 0 else wkr_s
                            w_ap = w_s[:, hcc, (ct - 1) * P:ct * P]
                        nc.tensor.matmul(
                            ps,
                            lhsT=w_ap,
                            rhs=xt[:, hcc, sb * 512:(sb + 1) * 512],
                            start=(hcc == 0), stop=(hcc == 7))
                    nc.vector.tensor_scalar_add(
                        dst[:, ct, sb * 512:(sb + 1) * 512], ps,
                        bqk[:, 4 * wi + ct:4 * wi + ct + 1])

        def emit_v():
            for st in range(8):
                ps = psc.tile([P, 512], f32, tag="psc")
                for hcc in range(8):
                    nc.tensor.matmul(
                        ps,
                        lhsT=xt[:, hcc, st * P:(st + 1) * P],
                        rhs=wv_s[:, hcc, :],
                        start=(hcc == 0), stop=(hcc == 7))
                nc.vector.tensor_add(
                    vaug[:, st, :, 0:D],
                    ps.rearrange("p (h d) -> p h d", h=HG),
                    bvb.rearrange("p (h d) -> p h d", h=HG))

        pt_of = {}

        def emit_scores(h):
            ct, pb = h // 2, (h % 2) * D
            ptile = ptpool.tile([P, 8, S], f16, tag="pt")
            pt_of[h] = ptile
            for kt_i in range(8):
                ps_s = pss.tile([P, S], f32, tag="pss")
                for qb in range(2):
                    nc.tensor.matmul(
                        ps_s[:, qb * 512:(qb + 1) * 512],
                        lhsT=kt[pb:pb + D, ct, kt_i * P:(kt_i + 1) * P],
                        rhs=qt[pb:pb + D, ct, qb * 512:(qb + 1) * 512],
                        start=True, stop=True)
                nc.scalar.activation(ptile[:, kt_i, :], ps_s, EXP, scale=0.125)

        def emit_ctx(h):
            ptile = pt_of.pop(h)
            for qb in range(2):
                ps_c = psc.tile([P, 512], f32, tag="psc")
                for kt_i in range(8):
                    nc.tensor.matmul(
                        ps_c[0:D + 1, :],
                        lhsT=vaug[:, kt_i, h, :],
                        rhs=ptile[:, kt_i, qb * 512:(qb + 1) * 512],
                        start=(kt_i == 0), stop=(kt_i == 7))
                cts = ctspool.tile([D + 1, 512], f16, tag="cts")
                nc.vector.tensor_copy(cts, ps_c[0:D + 1, :])
                # stride 66 elements keeps each j-slice 4-byte aligned in PSUM
                ps_t = pst.tile([P, 4, D + 2], f16, tag="pxt")
                for j in range(4):
                    nc.tensor.transpose(
                        ps_t[:, j, 0:D + 1], cts[:, j * P:(j + 1) * P],
                        ident[0:D + 1, 0:D + 1])
                r = rpool.tile([P, 4], f32, tag="r")
                nc.vector.reciprocal(r, ps_t[:, :, D])
                for j in range(4):
                    nc.vector.tensor_scalar_mul(
                        out_s[:, qb * 4 + j, h * D:(h + 1) * D],
                        ps_t[:, j, 0:D], r[:, j:j + 1])

        out_r = out_d.rearrange("(q p) n -> p q n", p=P)

        # software-pipelined head loop: exp(h) runs on ACT while the PE does
        # V / next-ct projections / ctx(h-1); scores stay one head ahead.
        emit_qtkt(0)
        emit_scores(0)
        emit_v()
        emit_scores(1)
        emit_ctx(0)
        for ct in range(1, 4):
            emit_qtkt(ct)
            emit_scores(2 * ct)
            emit_ctx(2 * ct - 1)
            if ct == 2:
                # heads 0-3 done: first half of the output columns
                nc.sync.dma_start(out=out_r[:, :, 0:4 * D],
                                  in_=out_s[:, :, 0:4 * D])
            emit_scores(2 * ct + 1)
            emit_ctx(2 * ct)
        emit_ctx(7)
        nc.sync.dma_start(out=out_r[:, :, 4 * D:8 * D],
                          in_=out_s[:, :, 4 * D:8 * D])

    _split_excess_waits(nc, mybir)
    return nc


def _get_nc():
    if "nc" not in _CACHE:
        _CACHE["nc"] = _build()
    return _CACHE["nc"]


def _in_maps(inputs):
    hs = np.ascontiguousarray(np.asarray(inputs["hidden_states"], dtype=np.float32))
    Wq = np.asarray(inputs["Wq"], dtype=np.float32)
    Wk = np.asarray(inputs["Wk"], dtype=np.float32)
    Wv = np.asarray(inputs["Wv"], dtype=np.float32)
    bq = np.asarray(inputs["bq"], dtype=np.float32)
    bk = np.asarray(inputs["bk"], dtype=np.float32)
    bv = np.asarray(inputs["bv"], dtype=np.float32)
    maps = []
    for c in range(NCORES):
        b, g = c // 2, c % 2
        sl = slice(g * CW, (g + 1) * CW)
        wq_sl = Wq[:, sl].astype(np.float16)
        wk_sl = Wk[:, sl].astype(np.float16)
        m = {
            "x": np.ascontiguousarray(hs[b].T).astype(np.float16),
            "wqk0": np.ascontiguousarray(
                np.concatenate([wq_sl[:, 0:P], wk_sl[:, 0:P]], axis=1)),
            "wqr": np.ascontiguousarray(wq_sl[:, P:CW]),
            "wkr": np.ascontiguousarray(wk_sl[:, P:CW]),
            "wv": np.ascontiguousarray(Wv[:, sl]).astype(np.float16),
            "bqk": np.ascontiguousarray(np.concatenate(
                [bq[sl].reshape(4, P).T, bk[sl].reshape(4, P).T], axis=1)),
            "bvb": np.ascontiguousarray(np.broadcast_to(bv[sl], (P, CW))),
            "id16": np.eye(P, dtype=np.float16),
        }
        maps.append(m)
    return maps


def run(inputs, **spmd_kwargs):
    """Run on 8 cores; returns (full_output, BassKernelResults)."""
    from concourse.bass_utils import run_bass_kernel_spmd
    nc = _get_nc()
    res = run_bass_kernel_spmd(nc, _in_maps(inputs), list(range(NCORES)),
                               **spmd_kwargs)
    out = np.empty((B, S, H), dtype=np.float32)
    for c in range(NCORES):
        b, g = c // 2, c % 2
        out[b, :, g * CW:(g + 1) * CW] = res.results[c]["out"].astype(np.float32)
    return out, res


def kernel(**inputs):
    out, _ = run(inputs)
    return out


# revision 9
# speedup vs baseline: 1.0442x; 1.0442x over previous
"""BERT self-attention (B=4, S=1024, H=1024, 16 heads, d=64) on 8 TRN2 cores.

Sharding: core c = b*2 + g handles batch b and head-group g (8 heads, 512
output columns).  No cross-core communication.

Baseline was 140.2us with PE (matmul pipe) busy 116.4us, a 17us dead start
(serialized DMA issues on 2 queues, W ct0 slices queued behind misc loads)
and a 12.6us tail (drain of ~29 DMA semaphores + fp32 transposes).

This version:
  1. Startup: input DMAs spread over 3 HWDGE queues (sync/scalar/vector)
     ordered by first use; wq|wk ct0 packed into one 512B-row tensor so the
     first QTKT chain starts ~9-10us in.  Six warmup matmuls on a memset
     tile run during the DMA fill so the PE p-state ramp (0.65/1.2GHz ->
     2.4GHz over ~3us) is spent before real work; EXP activation table is
     preloaded off the critical path.
  2. Tail: bq/bk packed into one tensor, context staged into a persistent
     [128, 8, 512] SBUF tile and written with 2 output DMAs (512B runs,
     1x DMA cost) instead of 8, fewer semaphores for the end drain.
  3. f16 normalization path: ctx~^T copied to f16, PE transpose in f16
     (1 cycle/row vs 2 for fp32), f16 PSUM transpose output lets the DVE
     scalar-mul run in 2x mode.

Per-core dataflow (matmul inputs fp16, accumulation fp32 PSUM):
  scores^T[k, q] = K_h^T.T @ Q_h^T (exp on ACT, 1/8 scale folded in); Vaug
  carries a ones column so ctx~^T = Vaug^T P^T also yields softmax
  denominators; PE-transpose back to [q, d+1], per-partition reciprocal *
  tensor_scalar_mul into the staging tile.  Software-pipelined head loop
  keeps scores one head ahead of ctx so the ACT exp stream (~71us) hides
  under PE work (~113us busy).
"""

import numpy as np

B, S, H = 4, 1024, 1024
NH, D = 16, 64
NCORES = 8
HG = NH // 2        # heads per core
CW = HG * D         # output columns per core (512)
P = 128             # partitions

_CACHE = {}


def _split_excess_waits(nc, mybir):
    """Walrus codegen allows 1 sync-wait per instruction (2 for
    EventSemaphore); Tile's tail drain (and some matmuls) carry more.
    Move the excess onto NoOp carriers inserted just before, same engine."""
    for f in nc.m.functions:
        for bb in f.blocks:
            new_insts, changed = [], False
            for inst in bb.instructions:
                si = inst.sync_info
                cap = 2 if inst.opcode == "EventSemaphore" else 1
                if si is not None and si.on_wait and len(si.on_wait) > cap:
                    waits = list(si.on_wait)
                    for i, w in enumerate(waits[:-cap]):
                        nop = mybir.InstNoOp(
                            name=f"{inst.name}-wsplit{i}",
                            engine=inst.engine,
                            sync_info=mybir.SyncInfo(on_wait=[w], on_update=[]),
                            bass_nofuse=True,
                        )
                        nc.register_instruction(nop, overwrite=True)
                        new_insts.append(nop)
                    inst.sync_info = mybir.SyncInfo(
                        on_wait=waits[-cap:],
                        on_update=list(si.on_update or []))
                    changed = True
                new_insts.append(inst)
            if changed:
                bb.instructions = new_insts


def _build():
    import concourse.bass as bass
    import concourse.mybir as mybir
    import concourse.tile as tile
    from contextlib import ExitStack

    f32 = mybir.dt.float32
    f16 = mybir.dt.float16
    EXP = mybir.ActivationFunctionType.Exp

    nc = bass.Bass()
    x_d = nc.dram_tensor("x", [H, S], f16, kind="ExternalInput")      # X^T
    wqk0_d = nc.dram_tensor("wqk0", [H, 2 * P], f16, kind="ExternalInput")
    wqr_d = nc.dram_tensor("wqr", [H, 3 * P], f16, kind="ExternalInput")
    wkr_d = nc.dram_tensor("wkr", [H, 3 * P], f16, kind="ExternalInput")
    wv_d = nc.dram_tensor("wv", [H, CW], f16, kind="ExternalInput")
    bqk_d = nc.dram_tensor("bqk", [P, 8], f32, kind="ExternalInput")
    bvb_d = nc.dram_tensor("bvb", [P, CW], f32, kind="ExternalInput")
    id16_d = nc.dram_tensor("id16", [P, P], f16, kind="ExternalInput")
    out_d = nc.dram_tensor("out", [S, CW], f16, kind="ExternalOutput")

    with tile.TileContext(nc) as tc, ExitStack() as ctx:
        persist = ctx.enter_context(tc.tile_pool(name="persist", bufs=1))
        ptpool = ctx.enter_context(tc.tile_pool(name="ptpool", bufs=2))
        ctspool = ctx.enter_context(tc.tile_pool(name="ctspool", bufs=2))
        rpool = ctx.enter_context(tc.tile_pool(name="rpool", bufs=4))
        pss = ctx.enter_context(tc.tile_pool(name="pss", bufs=2, space="PSUM"))
        psc = ctx.enter_context(tc.tile_pool(name="psc", bufs=2, space="PSUM"))
        pst = ctx.enter_context(tc.tile_pool(name="pst", bufs=2, space="PSUM"))

        wqk0_s = persist.tile([P, 8, 2 * P], f16, tag="wqk0")
        wqr_s = persist.tile([P, 8, 3 * P], f16, tag="wqr")
        wkr_s = persist.tile([P, 8, 3 * P], f16, tag="wkr")
        wv_s = persist.tile([P, 8, CW], f16, tag="wv")
        ident = persist.tile([P, P], f16, tag="ident")
        xt = persist.tile([P, 8, S], f16, tag="xt")          # X^T [h, hc, s]
        qt = persist.tile([P, 4, S], f16, tag="qt")          # Q^T [col, ct, s]
        kt = persist.tile([P, 4, S], f16, tag="kt")          # K^T
        vaug = persist.tile([P, 8, HG, D + 1], f16, tag="vaug")  # V + ones col
        bqk = persist.tile([P, 8], f32, tag="bqk")
        bvb = persist.tile([P, CW], f32, tag="bvb")          # bv broadcast
        ones1 = persist.tile([1, P], f32, tag="ones1")
        onesf = persist.tile([P, 8, HG], f32, tag="onesf")
        warm = persist.tile([P, 512], f16, tag="warm")
        actw = persist.tile([P, 1], f32, tag="actw")
        out_s = persist.tile([P, 8, CW], f16, tag="outs")    # ctx staging

        # ---- warmup: memset a tile, then 7 matmuls so the PE p-state
        # ramp completes while the input DMAs stream in ----
        nc.gpsimd.memset(warm, 0.0625)
        for _ in range(7):
            ps = psc.tile([P, 512], f32, tag="psc")
            nc.tensor.matmul(ps, lhsT=warm[:, 0:P], rhs=warm,
                             start=True, stop=True)

        # ---- input DMAs: 3 queues, ordered by first use; big late loads
        # (wv, wqr, wkr) go behind the X chunks so they can't starve them ----
        x_r = x_d.rearrange("(c p) s -> p c s", p=P)
        wqr_r = wqr_d.rearrange("(c p) n -> p c n", p=P)
        wkr_r = wkr_d.rearrange("(c p) n -> p c n", p=P)
        # sync: packed wq|wk ct0 (first chain's stationary), first X chunks
        nc.sync.dma_start(out=wqk0_s,
                          in_=wqk0_d.rearrange("(c p) n -> p c n", p=P))
        for hc in (0, 1, 2, 3):
            nc.sync.dma_start(out=xt[:, hc, :], in_=x_r[:, hc, :])
        # scalar: remaining X chunks, then wv/wqr/wkr, then EXP table
        # preload; ACT's sequencer is free before the exp stream starts
        for hc in (4, 5, 6, 7):
            nc.scalar.dma_start(out=xt[:, hc, :], in_=x_r[:, hc, :])
        nc.scalar.dma_start(out=wv_s, in_=wv_d.rearrange("(c p) n -> p c n", p=P))
        nc.scalar.dma_start(out=wqr_s, in_=wqr_r)
        nc.scalar.dma_start(out=wkr_s, in_=wkr_r)
        nc.scalar.activation(actw, warm[:, 0:1], EXP, scale=1.0)
        # gpsimd (SWDGE): small misc tensors
        nc.gpsimd.dma_start(out=bqk, in_=bqk_d[:, :])
        nc.gpsimd.dma_start(out=bvb, in_=bvb_d[:, :])
        nc.gpsimd.dma_start(out=ident, in_=id16_d[:, :])

        nc.vector.memset(onesf, 1.0)
        nc.vector.tensor_copy(vaug[:, :, :, D], onesf)
        nc.vector.memset(ones1, 1.0)

        def emit_proj_chain(ct, wi, sb):
            """One QTKT chain: Q (wi=0) or K (wi=1), S-half sb."""
            dst = qt if wi == 0 else kt
            ps = psc.tile([P, 512], f32, tag="psc")
            for hcc in range(8):
                if ct == 0:
                    w_ap = wqk0_s[:, hcc, wi * P:(wi + 1) * P]
                else:
                    w_s = wqr_s if wi == 0 else wkr_s
                    w_ap = w_s[:, hcc, (ct - 1) * P:ct * P]
                nc.tensor.matmul(
                    ps,
                    lhsT=w_ap,
                    rhs=xt[:, hcc, sb * 512:(sb + 1) * 512],
                    start=(hcc == 0), stop=(hcc == 7))
            nc.vector.tensor_scalar_add(
                dst[:, ct, sb * 512:(sb + 1) * 512], ps,
                bqk[:, 4 * wi + ct:4 * wi + ct + 1])

        def emit_v_chain(st):
            ps = psc.tile([P, 512], f32, tag="psc")
            for hcc in range(8):
                nc.tensor.matmul(
                    ps,
                    lhsT=xt[:, hcc, st * P:(st + 1) * P],
                    rhs=wv_s[:, hcc, :],
                    start=(hcc == 0), stop=(hcc == 7))
            nc.vector.tensor_add(
                vaug[:, st, :, 0:D],
                ps.rearrange("p (h d) -> p h d", h=HG),
                bvb.rearrange("p (h d) -> p h d", h=HG))

        pt_of = {}

        def emit_score_chunk(h, kt_i):
            """Scores^T for 128 keys x all 1024 q of head h, plus its exp."""
            ct, pb = h // 2, (h % 2) * D
            if kt_i == 0:
                ptile = ptpool.tile([P, 8, S], f16, tag="pt")
                pt_of[h] = ptile
            ptile = pt_of[h]
            ps_s = pss.tile([P, S], f32, tag="pss")
            for qb in range(2):
                nc.tensor.matmul(
                    ps_s[:, qb * 512:(qb + 1) * 512],
                    lhsT=kt[pb:pb + D, ct, kt_i * P:(kt_i + 1) * P],
                    rhs=qt[pb:pb + D, ct, qb * 512:(qb + 1) * 512],
                    start=True, stop=True)
            nc.scalar.activation(ptile[:, kt_i, :], ps_s, EXP, scale=0.125)

        def emit_ctx_unit(h, qb):
            ptile = pt_of[h]
            ps_c = psc.tile([P, 512], f32, tag="psc")
            for kt_i in range(8):
                nc.tensor.matmul(
                    ps_c[0:D + 1, :],
                    lhsT=vaug[:, kt_i, h, :],
                    rhs=ptile[:, kt_i, qb * 512:(qb + 1) * 512],
                    start=(kt_i == 0), stop=(kt_i == 7))
            cts = ctspool.tile([D + 1, 512], f16, tag="cts")
            nc.vector.tensor_copy(cts, ps_c[0:D + 1, :])
            # stride 66 elements keeps each j-slice 4-byte aligned in PSUM
            ps_t = pst.tile([P, 4, D + 2], f16, tag="pxt")
            for j in range(4):
                nc.tensor.transpose(
                    ps_t[:, j, 0:D + 1], cts[:, j * P:(j + 1) * P],
                    ident[0:D + 1, 0:D + 1])
            r = rpool.tile([P, 4], f32, tag="r")
            nc.vector.reciprocal(r, ps_t[:, :, D])
            for j in range(4):
                nc.vector.tensor_scalar_mul(
                    out_s[:, qb * 4 + j, h * D:(h + 1) * D],
                    ps_t[:, j, 0:D], r[:, j:j + 1])
            if qb == 1:
                pt_of.pop(h)

        out_r = out_d.rearrange("(q p) n -> p q n", p=P)

        # Block-interleaved schedule: each block h emits the 8 score chunks
        # of head h spread between filler chains (projections, V, ctx of the
        # previous head) so the PE never has to wait for ACT to drain a
        # scores PSUM tile (pss bufs=2 lets scores run only 2 chunks ahead
        # of the exp stream).  Fillers are front-loaded per the data deps:
        # qt/kt(ct) before scores(2ct), all V before ctx(0), exp(h) before
        # ctx(h); projection chains are spread toward the late blocks to
        # keep the PE/ACT ratio per block above 1.
        FILL = {
            0: [("v", st) for st in range(8)],
            1: [("p", 1, 0, 0), ("p", 1, 0, 1), ("p", 1, 1, 0), ("p", 1, 1, 1),
                ("c", 0, 0), ("c", 0, 1)],
            2: [("p", 2, 0, 0), ("c", 1, 0), ("c", 1, 1), ("p", 2, 0, 1)],
            3: [("p", 2, 1, 0), ("c", 2, 0), ("c", 2, 1), ("p", 2, 1, 1)],
            4: [("p", 3, 0, 0), ("c", 3, 0), ("c", 3, 1), ("p", 3, 0, 1)],
            5: [("p", 3, 1, 0), ("c", 4, 0), ("c", 4, 1), ("p", 3, 1, 1)],
            6: [("c", 5, 0), ("c", 5, 1)],
            7: [("c", 6, 0), ("c", 6, 1)],
        }

        def emit_fill(u):
            if u[0] == "v":
                emit_v_chain(u[1])
            elif u[0] == "p":
                emit_proj_chain(u[1], u[2], u[3])
            else:
                emit_ctx_unit(u[1], u[2])

        for wi in range(2):
            for sb in range(2):
                emit_proj_chain(0, wi, sb)
        for h in range(8):
            fillers = list(FILL[h])
            nf, done = len(fillers), 0
            for kt_i in range(8):
                emit_score_chunk(h, kt_i)
                while done < ((kt_i + 1) * nf) // 8:
                    emit_fill(fillers.pop(0))
                    done += 1
            for u in fillers:
                emit_fill(u)
            if h == 4:
                # heads 0-3 done: first half of the output columns
                nc.sync.dma_start(out=out_r[:, :, 0:4 * D],
                                  in_=out_s[:, :, 0:4 * D])
        emit_ctx_unit(7, 0)
        emit_ctx_unit(7, 1)
        nc.sync.dma_start(out=out_r[:, :, 4 * D:8 * D],
                          in_=out_s[:, :, 4 * D:8 * D])

    _split_excess_waits(nc, mybir)
    return nc


def _get_nc():
    if "nc" not in _CACHE:
        _CACHE["nc"] = _build()
    return _CACHE["nc"]


def _in_maps(inputs):
    hs = np.ascontiguousarray(np.asarray(inputs["hidden_states"], dtype=np.float32))
    Wq = np.asarray(inputs["Wq"], dtype=np.float32)
    Wk = np.asarray(inputs["Wk"], dtype=np.float32)
    Wv = np.asarray(inputs["Wv"], dtype=np.float32)
    bq = np.asarray(inputs["bq"], dtype=np.float32)
    bk = np.asarray(inputs["bk"], dtype=np.float32)
    bv = np.asarray(inputs["bv"], dtype=np.float32)
    maps = []
    for c in range(NCORES):
        b, g = c // 2, c % 2
        sl = slice(g * CW, (g + 1) * CW)
        wq_sl = Wq[:, sl].astype(np.float16)
        wk_sl = Wk[:, sl].astype(np.float16)
        m = {
            "x": np.ascontiguousarray(hs[b].T).astype(np.float16),
            "wqk0": np.ascontiguousarray(
                np.concatenate([wq_sl[:, 0:P], wk_sl[:, 0:P]], axis=1)),
            "wqr": np.ascontiguousarray(wq_sl[:, P:CW]),
            "wkr": np.ascontiguousarray(wk_sl[:, P:CW]),
            "wv": np.ascontiguousarray(Wv[:, sl]).astype(np.float16),
            "bqk": np.ascontiguousarray(np.concatenate(
                [bq[sl].reshape(4, P).T, bk[sl].reshape(4, P).T], axis=1)),
            "bvb": np.ascontiguousarray(np.broadcast_to(bv[sl], (P, CW))),
            "id16": np.eye(P, dtype=np.float16),
        }
        maps.append(m)
    return maps


def run(inputs, **spmd_kwargs):
    """Run on 8 cores; returns (full_output, BassKernelResults)."""
    from concourse.bass_utils import run_bass_kernel_spmd
    nc = _get_nc()
    res = run_bass_kernel_spmd(nc, _in_maps(inputs), list(range(NCORES)),
                               **spmd_kwargs)
    out = np.empty((B, S, H), dtype=np.float32)
    for c in range(NCORES):
        b, g = c // 2, c % 2
        out[b, :, g * CW:(g + 1) * CW] = res.results[c]["out"].astype(np.float32)
    return out, res


def kernel(**inputs):
    out, _ = run(inputs)
    return out


# revision 11
# speedup vs baseline: 1.0503x; 1.0058x over previous
"""BERT self-attention (B=4, S=1024, H=1024, 16 heads, d=64) on 8 TRN2 cores.

Sharding: core c = b*2 + g handles batch b and head-group g (8 heads, 512
output columns).  No cross-core communication.

Baseline was 140.2us with PE (matmul pipe) busy 116.4us, a 17us dead start
(serialized DMA issues on 2 queues, W ct0 slices queued behind misc loads)
and a 12.6us tail (drain of ~29 DMA semaphores + fp32 transposes).

This version:
  1. Startup: input DMAs spread over 3 HWDGE queues (sync/scalar/vector)
     ordered by first use; wq|wk ct0 packed into one 512B-row tensor so the
     first QTKT chain starts ~9-10us in.  Six warmup matmuls on a memset
     tile run during the DMA fill so the PE p-state ramp (0.65/1.2GHz ->
     2.4GHz over ~3us) is spent before real work; EXP activation table is
     preloaded off the critical path.
  2. Tail: bq/bk packed into one tensor, context staged into a persistent
     [128, 8, 512] SBUF tile and written with 2 output DMAs (512B runs,
     1x DMA cost) instead of 8, fewer semaphores for the end drain.
  3. f16 normalization path: ctx~^T copied to f16, PE transpose in f16
     (1 cycle/row vs 2 for fp32), f16 PSUM transpose output lets the DVE
     scalar-mul run in 2x mode.

Per-core dataflow (matmul inputs fp16, accumulation fp32 PSUM):
  scores^T[k, q] = K_h^T.T @ Q_h^T (exp on ACT, 1/8 scale folded in); Vaug
  carries a ones column so ctx~^T = Vaug^T P^T also yields softmax
  denominators; PE-transpose back to [q, d+1], per-partition reciprocal *
  tensor_scalar_mul into the staging tile.  Software-pipelined head loop
  keeps scores one head ahead of ctx so the ACT exp stream (~71us) hides
  under PE work (~113us busy).
"""

import numpy as np

B, S, H = 4, 1024, 1024
NH, D = 16, 64
NCORES = 8
HG = NH // 2        # heads per core
CW = HG * D         # output columns per core (512)
P = 128             # partitions

_CACHE = {}


def _split_excess_waits(nc, mybir):
    """Walrus codegen allows 1 sync-wait per instruction (2 for
    EventSemaphore); Tile's tail drain (and some matmuls) carry more.
    Move the excess onto NoOp carriers inserted just before, same engine."""
    for f in nc.m.functions:
        for bb in f.blocks:
            new_insts, changed = [], False
            for inst in bb.instructions:
                si = inst.sync_info
                cap = 2 if inst.opcode == "EventSemaphore" else 1
                if si is not None and si.on_wait and len(si.on_wait) > cap:
                    waits = list(si.on_wait)
                    for i, w in enumerate(waits[:-cap]):
                        nop = mybir.InstNoOp(
                            name=f"{inst.name}-wsplit{i}",
                            engine=inst.engine,
                            sync_info=mybir.SyncInfo(on_wait=[w], on_update=[]),
                            bass_nofuse=True,
                        )
                        nc.register_instruction(nop, overwrite=True)
                        new_insts.append(nop)
                    inst.sync_info = mybir.SyncInfo(
                        on_wait=waits[-cap:],
                        on_update=list(si.on_update or []))
                    changed = True
                new_insts.append(inst)
            if changed:
                bb.instructions = new_insts


def _build():
    import concourse.bass as bass
    import concourse.mybir as mybir
    import concourse.tile as tile
    from contextlib import ExitStack

    f32 = mybir.dt.float32
    f16 = mybir.dt.float16
    EXP = mybir.ActivationFunctionType.Exp

    nc = bass.Bass()
    x_d = nc.dram_tensor("x", [H, S], f16, kind="ExternalInput")      # X^T
    wq0_d = nc.dram_tensor("wq0", [H, P], f16, kind="ExternalInput")
    wk0_d = nc.dram_tensor("wk0", [H, P], f16, kind="ExternalInput")
    wqr_d = nc.dram_tensor("wqr", [H, 3 * P], f16, kind="ExternalInput")
    wkr_d = nc.dram_tensor("wkr", [H, 3 * P], f16, kind="ExternalInput")
    wv_d = nc.dram_tensor("wv", [H, CW], f16, kind="ExternalInput")
    bqk_d = nc.dram_tensor("bqk", [P, 8], f32, kind="ExternalInput")
    bvb_d = nc.dram_tensor("bvb", [P, CW], f32, kind="ExternalInput")
    id16_d = nc.dram_tensor("id16", [P, P], f16, kind="ExternalInput")
    out_d = nc.dram_tensor("out", [S, CW], f16, kind="ExternalOutput")

    with tile.TileContext(nc) as tc, ExitStack() as ctx:
        persist = ctx.enter_context(tc.tile_pool(name="persist", bufs=1))
        ptpool = ctx.enter_context(tc.tile_pool(name="ptpool", bufs=2))
        ctspool = ctx.enter_context(tc.tile_pool(name="ctspool", bufs=2))
        rpool = ctx.enter_context(tc.tile_pool(name="rpool", bufs=4))
        pss = ctx.enter_context(tc.tile_pool(name="pss", bufs=2, space="PSUM"))
        psc = ctx.enter_context(tc.tile_pool(name="psc", bufs=2, space="PSUM"))
        pst = ctx.enter_context(tc.tile_pool(name="pst", bufs=2, space="PSUM"))

        wq0_s = persist.tile([P, 8, P], f16, tag="wq0")
        wk0_s = persist.tile([P, 8, P], f16, tag="wk0")
        wqr_s = persist.tile([P, 8, 3 * P], f16, tag="wqr")
        wkr_s = persist.tile([P, 8, 3 * P], f16, tag="wkr")
        wv_s = persist.tile([P, 8, CW], f16, tag="wv")
        ident = persist.tile([P, P], f16, tag="ident")
        xt = persist.tile([P, 8, S], f16, tag="xt")          # X^T [h, hc, s]
        qt = persist.tile([P, 4, S], f16, tag="qt")          # Q^T [col, ct, s]
        kt = persist.tile([P, 4, S], f16, tag="kt")          # K^T
        vaug = persist.tile([P, 8, HG, D + 1], f16, tag="vaug")  # V + ones col
        bqk = persist.tile([P, 8], f32, tag="bqk")
        bvb = persist.tile([P, CW], f32, tag="bvb")          # bv broadcast
        ones1 = persist.tile([1, P], f32, tag="ones1")
        onesf = persist.tile([P, 8, HG], f32, tag="onesf")
        warm = persist.tile([P, 512], f16, tag="warm")
        actw = persist.tile([P, 1], f32, tag="actw")
        out_s = persist.tile([P, 8, CW], f16, tag="outs")    # ctx staging

        # ---- warmup: memset a tile, then 7 matmuls so the PE p-state
        # ramp completes while the input DMAs stream in ----
        nc.gpsimd.memset(warm, 0.0625)
        for _ in range(10):
            ps = psc.tile([P, 512], f32, tag="psc")
            nc.tensor.matmul(ps, lhsT=warm[:, 0:P], rhs=warm,
                             start=True, stop=True)

        # ---- input DMAs: ordered by first use.  The first QTKT chain
        # needs wq0 + xt chunks in hcc order; X is spread over all three
        # queues, big late loads (wv, wqr, wkr) go last on scalar ----
        x_r = x_d.rearrange("(c p) s -> p c s", p=P)
        wqr_r = wqr_d.rearrange("(c p) n -> p c n", p=P)
        wkr_r = wkr_d.rearrange("(c p) n -> p c n", p=P)
        nc.sync.dma_start(out=wq0_s,
                          in_=wq0_d.rearrange("(c p) n -> p c n", p=P))
        for hc in (1, 3, 5):
            nc.sync.dma_start(out=xt[:, hc, :], in_=x_r[:, hc, :])
        for hc in (0, 2, 4):
            nc.scalar.dma_start(out=xt[:, hc, :], in_=x_r[:, hc, :])
        nc.scalar.dma_start(out=wk0_s,
                            in_=wk0_d.rearrange("(c p) n -> p c n", p=P))
        nc.scalar.dma_start(out=wv_s, in_=wv_d.rearrange("(c p) n -> p c n", p=P))
        nc.scalar.dma_start(out=wqr_s, in_=wqr_r)
        nc.scalar.dma_start(out=wkr_s, in_=wkr_r)
        nc.scalar.activation(actw, warm[:, 0:1], EXP, scale=1.0)
        # gpsimd (SWDGE): two X chunks and the small misc tensors
        nc.gpsimd.dma_start(out=xt[:, 6, :], in_=x_r[:, 6, :])
        nc.gpsimd.dma_start(out=xt[:, 7, :], in_=x_r[:, 7, :])
        nc.gpsimd.dma_start(out=bqk, in_=bqk_d[:, :])
        nc.gpsimd.dma_start(out=bvb, in_=bvb_d[:, :])
        nc.gpsimd.dma_start(out=ident, in_=id16_d[:, :])

        nc.vector.memset(onesf, 1.0)
        nc.vector.tensor_copy(vaug[:, :, :, D], onesf)
        nc.vector.memset(ones1, 1.0)

        def emit_proj_chain(ct, wi, sb):
            """One QTKT chain: Q (wi=0) or K (wi=1), S-half sb."""
            dst = qt if wi == 0 else kt
            ps = psc.tile([P, 512], f32, tag="psc")
            for hcc in range(8):
                if ct == 0:
                    w_ap = (wq0_s if wi == 0 else wk0_s)[:, hcc, :]
                else:
                    w_s = wqr_s if wi == 0 else wkr_s
                    w_ap = w_s[:, hcc, (ct - 1) * P:ct * P]
                nc.tensor.matmul(
                    ps,
                    lhsT=w_ap,
                    rhs=xt[:, hcc, sb * 512:(sb + 1) * 512],
                    start=(hcc == 0), stop=(hcc == 7))
            nc.vector.tensor_scalar_add(
                dst[:, ct, sb * 512:(sb + 1) * 512], ps,
                bqk[:, 4 * wi + ct:4 * wi + ct + 1])

        def emit_v_chain(st):
            ps = psc.tile([P, 512], f32, tag="psc")
            for hcc in range(8):
                nc.tensor.matmul(
                    ps,
                    lhsT=xt[:, hcc, st * P:(st + 1) * P],
                    rhs=wv_s[:, hcc, :],
                    start=(hcc == 0), stop=(hcc == 7))
            nc.vector.tensor_add(
                vaug[:, st, :, 0:D],
                ps.rearrange("p (h d) -> p h d", h=HG),
                bvb.rearrange("p (h d) -> p h d", h=HG))

        pt_of = {}

        def emit_score_chunk(h, kt_i):
            """Scores^T for 128 keys x all 1024 q of head h, plus its exp."""
            ct, pb = h // 2, (h % 2) * D
            if kt_i == 0:
                ptile = ptpool.tile([P, 8, S], f16, tag="pt")
                pt_of[h] = ptile
            ptile = pt_of[h]
            ps_s = pss.tile([P, S], f32, tag="pss")
            for qb in range(2):
                nc.tensor.matmul(
                    ps_s[:, qb * 512:(qb + 1) * 512],
                    lhsT=kt[pb:pb + D, ct, kt_i * P:(kt_i + 1) * P],
                    rhs=qt[pb:pb + D, ct, qb * 512:(qb + 1) * 512],
                    start=True, stop=True)
            nc.scalar.activation(ptile[:, kt_i, :], ps_s, EXP, scale=0.125)

        def emit_ctx_unit(h, qb):
            ptile = pt_of[h]
            ps_c = psc.tile([P, 512], f32, tag="psc")
            for kt_i in range(8):
                nc.tensor.matmul(
                    ps_c[0:D + 1, :],
                    lhsT=vaug[:, kt_i, h, :],
                    rhs=ptile[:, kt_i, qb * 512:(qb + 1) * 512],
                    start=(kt_i == 0), stop=(kt_i == 7))
            cts = ctspool.tile([D + 1, 512], f16, tag="cts")
            nc.vector.tensor_copy(cts, ps_c[0:D + 1, :])
            # stride 66 elements keeps each j-slice 4-byte aligned in PSUM
            ps_t = pst.tile([P, 4, D + 2], f16, tag="pxt")
            for j in range(4):
                nc.tensor.transpose(
                    ps_t[:, j, 0:D + 1], cts[:, j * P:(j + 1) * P],
                    ident[0:D + 1, 0:D + 1])
            r = rpool.tile([P, 4], f32, tag="r")
            nc.vector.reciprocal(r, ps_t[:, :, D])
            for j in range(4):
                nc.vector.tensor_scalar_mul(
                    out_s[:, qb * 4 + j, h * D:(h + 1) * D],
                    ps_t[:, j, 0:D], r[:, j:j + 1])
            if qb == 1:
                pt_of.pop(h)

        out_r = out_d.rearrange("(q p) n -> p q n", p=P)

        # Block-interleaved schedule: each block h emits the 8 score chunks
        # of head h spread between filler chains (projections, V, ctx of the
        # previous head) so the PE never has to wait for ACT to drain a
        # scores PSUM tile (pss bufs=2 lets scores run only 2 chunks ahead
        # of the exp stream).  Fillers are front-loaded per the data deps:
        # qt/kt(ct) before scores(2ct), all V before ctx(0), exp(h) before
        # ctx(h); projection chains are spread toward the late blocks to
        # keep the PE/ACT ratio per block above 1.
        FILL = {
            0: [("v", st) for st in range(8)],
            1: [("p", 1, 0, 0), ("p", 1, 0, 1), ("p", 1, 1, 0), ("p", 1, 1, 1),
                ("c", 0, 0), ("c", 0, 1)],
            2: [("p", 2, 0, 0), ("c", 1, 0), ("c", 1, 1), ("p", 2, 0, 1)],
            3: [("p", 2, 1, 0), ("c", 2, 0), ("c", 2, 1), ("p", 2, 1, 1)],
            4: [("p", 3, 0, 0), ("c", 3, 0), ("c", 3, 1), ("p", 3, 0, 1)],
            5: [("p", 3, 1, 0), ("c", 4, 0), ("c", 4, 1), ("p", 3, 1, 1)],
            6: [("c", 5, 0), ("c", 5, 1)],
            7: [("c", 6, 0), ("c", 6, 1)],
        }

        def emit_fill(u):
            if u[0] == "v":
                emit_v_chain(u[1])
            elif u[0] == "p":
                emit_proj_chain(u[1], u[2], u[3])
            else:
                emit_ctx_unit(u[1], u[2])

        for wi in range(2):
            for sb in range(2):
                emit_proj_chain(0, wi, sb)
        for h in range(8):
            fillers = list(FILL[h])
            nf, done = len(fillers), 0
            for kt_i in range(8):
                emit_score_chunk(h, kt_i)
                while done < ((kt_i + 1) * nf) // 8:
                    emit_fill(fillers.pop(0))
                    done += 1
            for u in fillers:
                emit_fill(u)
            if h == 4:
                # heads 0-3 done: first half of the output columns
                nc.sync.dma_start(out=out_r[:, :, 0:4 * D],
                                  in_=out_s[:, :, 0:4 * D])
        nc.sync.dma_start(out=out_r[:, :, 4 * D:7 * D],
                          in_=out_s[:, :, 4 * D:7 * D])
        emit_ctx_unit(7, 0)
        emit_ctx_unit(7, 1)
        nc.sync.dma_start(out=out_r[:, :, 7 * D:8 * D],
                          in_=out_s[:, :, 7 * D:8 * D])

    _split_excess_waits(nc, mybir)
    return nc


def _get_nc():
    if "nc" not in _CACHE:
        _CACHE["nc"] = _build()
    return _CACHE["nc"]


def _in_maps(inputs):
    hs = np.ascontiguousarray(np.asarray(inputs["hidden_states"], dtype=np.float32))
    Wq = np.asarray(inputs["Wq"], dtype=np.float32)
    Wk = np.asarray(inputs["Wk"], dtype=np.float32)
    Wv = np.asarray(inputs["Wv"], dtype=np.float32)
    bq = np.asarray(inputs["bq"], dtype=np.float32)
    bk = np.asarray(inputs["bk"], dtype=np.float32)
    bv = np.asarray(inputs["bv"], dtype=np.float32)
    maps = []
    for c in range(NCORES):
        b, g = c // 2, c % 2
        sl = slice(g * CW, (g + 1) * CW)
        wq_sl = Wq[:, sl].astype(np.float16)
        wk_sl = Wk[:, sl].astype(np.float16)
        m = {
            "x": np.ascontiguousarray(hs[b].T).astype(np.float16),
            "wq0": np.ascontiguousarray(wq_sl[:, 0:P]),
            "wk0": np.ascontiguousarray(wk_sl[:, 0:P]),
            "wqr": np.ascontiguousarray(wq_sl[:, P:CW]),
            "wkr": np.ascontiguousarray(wk_sl[:, P:CW]),
            "wv": np.ascontiguousarray(Wv[:, sl]).astype(np.float16),
            "bqk": np.ascontiguousarray(np.concatenate(
                [bq[sl].reshape(4, P).T, bk[sl].reshape(4, P).T], axis=1)),
            "bvb": np.ascontiguousarray(np.broadcast_to(bv[sl], (P, CW))),
            "id16": np.eye(P, dtype=np.float16),
        }
        maps.append(m)
    return maps


def run(inputs, **spmd_kwargs):
    """Run on 8 cores; returns (full_output, BassKernelResults)."""
    from concourse.bass_utils import run_bass_kernel_spmd
    nc = _get_nc()
    res = run_bass_kernel_spmd(nc, _in_maps(inputs), list(range(NCORES)),
                               **spmd_kwargs)
    out = np.empty((B, S, H), dtype=np.float32)
    for c in range(NCORES):
        b, g = c // 2, c % 2
        out[b, :, g * CW:(g + 1) * CW] = res.results[c]["out"].astype(np.float32)
    return out, res


def kernel(**inputs):
    out, _ = run(inputs)
    return out
